# revision 26
# baseline (speedup 1.0000x reference)
"""Correntropy loss on 8 Trainium2 NeuronCores — centered-fp8 staging,
PE-subtract + ACT/DVE squares.

Reference math (all f32):
    t = (target - 0.5) * 2 ; o = (output - 0.5) * 2
    cost = mean(1 - exp(-sigma * (o - t)^2)),  sigma = 1/1000
Since o - t == 2*(output - target):
    cost = mean(1 - exp(-c * w)),  w = (output - target)^2,  c = 4*sigma

The kernel is HBM-bandwidth-bound; the rel-err budget (2e-2) is far
above f32 staging needs, so the host stages both tensors as CENTERED
fp8-e4m3 (q = fp8(x - 0.5)): 1/4 the bytes of f32.  Centering halves
e4m3's ulp over the data range.  Measured on the real key-0 data:
centered fp8 + 1-term series -> rel err 1.6e-3 (gate is 2e-2).

Device per core (row shard 8192 x 1000, folded to [128, 64000] cols):
    d = qo - qt  exactly, then S1 = sum(d^2), via two parallel routes:
  * PE route (~2/3 of cols): one DoubleRow fp8 matmul per 512-col chunk with
    stationary [I | -I] computes d into PSUM f32 exactly (2 rows/cyc,
    ~0.42 ns/col, weight reloads hidden).  ACT consumes 2048-col PSUM
    groups (4 banks) with Square-in-place + f32 accumulator
    (~1.17 ns/col incl. per-group accumulator read); groups ping-pong
    across the 8 PSUM banks.
  * DVE route (~1/3): tensor_sub fp8,fp8->fp8 (1 cyc/col) then
    scalar_tensor_tensor d*d with f32 accum (1 cyc/col); subs issued
    two slots ahead of their squares.  d and the dead w output stay
    fp8 to minimise SBUF traffic; the fp8 rounding of d measurably
    CANCELS part of the staging bias (1.9e-3 -> 1.6e-3 on key-0 data).
Host reduces everything in f64 and applies cost ~= c*S1/N (dropping
the -c^2/2*S2 series term: 8e-4 relative, inside the budget).

Schedule: 10 DMA tiles; small first tiles so ACT/DVE start ~10 us in
(the first DMA has ~7.5 us cold latency after the ~7 us framework
preamble); a dummy 1-col activation prefetches the Square LUT; the
last tile is PE-only so the post-final-DMA chain is short.  DMA
streams at ~420 GB/s (~42 us); DVE ~50 us busy is the critical
engine, ACT ~45 us; measured 66 us end to end.
"""

import numpy as np
import ml_dtypes

import concourse.bacc as bacc
import concourse.mybir as mybir
import concourse.tile as tile
from concourse.bass_utils import run_bass_kernel_spmd

N_CORES = 8
ROWS = 65536
COLS = 1000
ROWS_PER_CORE = ROWS // N_CORES  # 8192
P = 128  # SBUF partitions
TOTAL = ROWS_PER_CORE * COLS // P  # 64000 cols per operand per partition

GW = 2048  # ACT consumes PSUM in groups of <= GW cols (4 banks)
CW = 512  # one matmul / PSUM bank worth of cols
# Per DMA tile: (free_cols, dve_cols, reserved, n_pe_groups, n_dve_slices)
# pe cols = free - dve, consumed in GW-wide groups (last one may be partial).
TILE_CFG = [
    (3072, 1024, 0, 1, 1),
    (4096, 2048, 0, 1, 1),
    (8000, 3904, 0, 2, 1),
    (8000, 3904, 0, 2, 1),
    (8000, 3904, 0, 2, 1),
    (8000, 1856, 0, 3, 1),
    (8000, 1856, 0, 3, 1),
    (8000, 1856, 0, 3, 2),
    (8832, 1856, 0, 4, 2),
]
N_TILES = len(TILE_CFG)
assert sum(c[0] for c in TILE_CFG) == TOTAL
for fr, dw, gp, ng, _ in TILE_CFG:
    pe = fr - dw - gp
    assert pe >= 0 and ng == -(-pe // GW)

TILE_OFF = np.cumsum([0] + [c[0] for c in TILE_CFG]).tolist()

DVE_PIECES = []  # (tile, col_off, width)
for _t, (_fr, _dw, _gp, _g, _ns) in enumerate(TILE_CFG):
    for _k in range(_ns):
        step = _dw // _ns
        DVE_PIECES.append((_t, _k * step, step if _k < _ns - 1 else _dw - _k * step))
N_DVE = len(DVE_PIECES)
N_GRP = sum(c[3] for c in TILE_CFG)
ACC_COLS = N_DVE + N_GRP

F32 = mybir.dt.float32
BF16 = mybir.dt.bfloat16
FP8 = mybir.dt.float8e4


def _build():
    nc = bacc.Bacc()
    comb_p = nc.declare_dram_parameter(
        "combined", [2 * P, TOTAL], FP8, isOutput=False
    )
    wid_p = nc.declare_dram_parameter("wid", [P, 2 * P], FP8, isOutput=False)
    acc_p = nc.declare_dram_parameter("partial", [P, N_DVE], F32, isOutput=True)
    acc2_p = nc.declare_dram_parameter("partial2", [P, N_GRP], F32, isOutput=True)

    comb_v = comb_p[:].rearrange("(c p) m -> p c m", c=2, p=P)

    with tile.TileContext(nc) as tc:
        with (
            tc.tile_pool(name="io", bufs=1) as io_pool,
            tc.tile_pool(name="work", bufs=1) as work_pool,
            tc.tile_pool(name="accp", bufs=1) as acc_pool,
            tc.tile_pool(name="ps", bufs=1, space="PSUM") as ps_pool,
        ):
            acc = acc_pool.tile([P, N_DVE], F32)
            acc2 = acc_pool.tile([P, N_GRP], F32)
            stat = acc_pool.tile([P, 2 * P], FP8)
            nc.sync.dma_start(out=stat[:], in_=wid_p[:])
            stat_v = stat[:].rearrange("p (c m) -> p c m", c=2)

            Sq = mybir.ActivationFunctionType.Square
            M = mybir.AluOpType.mult

            # explicit zero bias AP (avoids framework const-AP loads) and a
            # dummy activation to prefetch the Square table set early
            zbias = acc_pool.tile([P, 1], F32)
            nc.vector.memset(zbias[:], 0.0)
            warm = acc_pool.tile([P, 1], F32)
            nc.vector.memset(warm[:], 0.0)
            nc.scalar.activation(warm[:], warm[:], Sq, bias=zbias[:])

            ab_tiles = {}

            def get_ab(t):
                if t not in ab_tiles:
                    fr = TILE_CFG[t][0]
                    ab = io_pool.tile([P, 2 * fr], FP8, tag=f"ab{fr}",
                                      bufs=(5 if fr == 8000 else 1))
                    o0 = TILE_OFF[t]
                    nc.sync.dma_start(
                        out=ab[:].rearrange("p (c m) -> p c m", c=2),
                        in_=comb_v[:, :, o0 : o0 + fr],
                    )
                    ab_tiles[t] = ab
                return ab_tiles[t]

            grp = 0
            dve_i = 0
            pending = []  # software-pipelined (d_tile, acc_col) awaiting stt

            def emit_stt():
                d, col = pending.pop(0)
                w = work_pool.tile([P, d.shape[1]], FP8, tag="w", bufs=3)
                nc.vector.scalar_tensor_tensor(
                    out=w[:], in0=d[:], scalar=1.0, in1=d[:],
                    op0=M, op1=M,
                    accum_out=acc[:, col : col + 1],
                )

            for t, (fr, dw, gp, ng, ns) in enumerate(TILE_CFG):
                ab = get_ab(t)
                ab_v = ab[:].rearrange("p (c m) -> p c m", c=2)

                # PE route: cols [dw+gp, fr) in GW-wide groups
                pe0 = dw + gp
                for g in range(ng):
                    base = pe0 + GW * g
                    gw = min(GW, fr - base)
                    pg = ps_pool.tile([P, GW], F32, tag=f"pg{grp % 2}")
                    for k in range(0, gw, CW):
                        cw = min(CW, gw - k)
                        nc.tensor.matmul(
                            pg[:, k : k + cw],
                            stat_v,
                            ab_v[:, :, base + k : base + k + cw],
                            start=True, stop=True,
                            perf_mode=mybir.MatmulPerfMode.DoubleRow,
                        )
                    # square in place in PSUM: no SBUF write at all
                    nc.scalar.activation(
                        pg[:, 0:gw], pg[:, 0:gw], Sq, bias=zbias[:],
                        accum_out=acc2[:, grp : grp + 1],
                    )
                    grp += 1

                # DVE route: cols [0, dw); sub issued 2 slots ahead of square
                while dve_i < N_DVE and DVE_PIECES[dve_i][0] == t:
                    _, off, z = DVE_PIECES[dve_i]
                    d = work_pool.tile([P, z], FP8, tag="d", bufs=3)
                    nc.vector.tensor_sub(
                        d[:], ab[:, off : off + z],
                        ab[:, fr + off : fr + off + z],
                    )
                    pending.append((d, dve_i))
                    if len(pending) >= 2:
                        emit_stt()
                    dve_i += 1
            while pending:
                emit_stt()

            nc.sync.dma_start(out=acc_p[:], in_=acc[:])
            nc.sync.dma_start(out=acc2_p[:], in_=acc2[:])
    nc.finalize()
    return nc


_NC = None


def _get_nc():
    global _NC
    if _NC is None:
        _NC = _build()
    return _NC


def _shard_inputs(output, target):
    output = np.asarray(output, dtype=np.float32)
    target = np.asarray(target, dtype=np.float32)
    qo = (output - np.float32(0.5)).astype(ml_dtypes.float8_e4m3)
    qt = (target - np.float32(0.5)).astype(ml_dtypes.float8_e4m3)

    idn = np.zeros((P, P), dtype=ml_dtypes.float8_e4m3)
    np.fill_diagonal(idn, 1.0)
    nidn = np.zeros((P, P), dtype=ml_dtypes.float8_e4m3)
    np.fill_diagonal(nidn, -1.0)
    wid = np.concatenate([idn, nidn], axis=1)  # [P, 2P]: I then -I

    in_maps = []
    for i in range(N_CORES):
        sl = slice(i * ROWS_PER_CORE, (i + 1) * ROWS_PER_CORE)
        o2 = qo[sl].reshape(P, TOTAL)
        t2 = qt[sl].reshape(P, TOTAL)
        comb = np.concatenate([o2[None], t2[None]], axis=0).reshape(2 * P, TOTAL)
        in_maps.append({"combined": comb, "wid": wid})
    return in_maps


def run_device(output, target, trace=False):
    """Returns (per-core (partial, wacc) arrays, BassKernelResults)."""
    in_maps = _shard_inputs(output, target)
    res = run_bass_kernel_spmd(_get_nc(), in_maps, list(range(N_CORES)), trace=trace)
    partials = [
        np.concatenate(
            [res.results[i]["partial"], res.results[i]["partial2"]], axis=1
        )
        for i in range(N_CORES)
    ]
    return partials, res


def _reduce(partials):
    s1 = 0.0
    for p in partials:
        s1 += p.astype(np.float64).sum()
    c = 4.0 * float(np.float32(1.0 / COLS))  # match reference's f32 sigma
    n = float(ROWS) * float(COLS)
    return np.array(c * s1 / n, dtype=np.float32)


def kernel(output, target):
    partials, _ = run_device(output, target)
    return _reduce(partials)
